# revision 25
# baseline (speedup 1.0000x reference)
"""Trainium2 Bass kernel for segment-reduce classifier.

Reference computation:
    local = relu(x @ Wloc.T)            # [L, 128]
    feats = local.reshape(-1, 30, 128).mean(1)   # [L/30, 128]
    out   = feats @ W.T                 # [L/30, 10]

Strategy (8 NeuronCores, data-parallel on rows):
  - Each core gets R = L/8 = 150000 rows, host-transposed, fp8(e3m4)-cast,
    packed as xt [128, 75000]: partitions 0-63 = x_shard[:75000].T ("A"
    half), partitions 64-127 = x_shard[75000:].T ("B" half).  fp8-e3m4
    halves the input DMA (9.6MB/core) vs fp16 and streams through the PE
    at 16-bit speed; 4 mantissa bits keep median rel err ~3.5e-3.
  - matmul1: lhsT = Wloc.T stacked twice [128, 128] fp16 (mixed dtype with
    fp8 rhs); two K=64 matmuls via PE row-groups (tile_position rows 0/64)
    produce localT [128enc, rows] 480-row chunks in PSUM.
  - relu PSUM -> SBUF fp16 with contiguous writes, j-scatter on the PSUM
    read side (PSUM fp32 source is 1 elem/lane/cyc regardless of stride),
    split between ScalarE and VectorE.  This evacuation is the kernel's
    hard floor: 150k lane-elems/core through the only two engines with a
    PSUM read port.
  - mean-pool + classifier fused: 30 accumulating matmuls per tile (one per
    within-segment offset j, rhs j-major) -> pooling is free PSUM
    accumulation. M=10 packed 4x into PE column-groups (tile_position
    (0,32s)); the 4 strips are summed on the host.  mm2 matmuls are
    emitted in sub-groups of ~8 interleaved between mm1 chunks: same-mode
    bursts amortize the PE tiling-mode-switch drain (measured 322ns/mm in
    1-bursts vs 61ns/mm in 8-bursts) while staying short enough not to
    starve the relu pipeline.
  - Tapered tile schedule: small first tiles so the PE starts ~2us after
    launch instead of waiting for a 2MB DMA; small last tile to shrink the
    drain tail.  Output DMA per tile, overlapped with compute.
"""

import numpy as np
import ml_dtypes

import concourse.bacc as bacc
import concourse.bass as bass
import concourse.tile as tile
from concourse import mybir
from concourse.bass_utils import run_bass_kernel_spmd

# Problem constants (hardcoded per harness contract)
L, D_IN, D_ENC, C, J = 1200000, 64, 128, 10, 30
N_CORES = 8
R = L // N_CORES          # rows per core = 150000
HALF = R // 2             # 75000 cols per half-stream
CH = 480                  # chunk rows (16 segments) per matmul
# tapered tile schedule (cols per half); sum = 75000, all %30 == 0
TFS = [960, 1920, 3840] + [7680] * 8 + [4440, 2400]
SEG_PER_CORE = R // J     # 5000
# j-subsets for the 4 PE column-group strips of the classifier matmul
J_SETS = [list(range(0, 8)), list(range(8, 16)),
          list(range(16, 23)), list(range(23, 30))]
# k-step groups for interleaved mm2 emission (pairs of k -> 8 matmuls each)
K_GROUPS = [(0, 2), (2, 4), (4, 6), (6, 8)]

_CACHE = {}


def _build_kernel():
    nc = bacc.Bacc("TRN2", target_bir_lowering=False, debug=False,
                   num_devices=N_CORES)
    f32, f16, f8 = mybir.dt.float32, mybir.dt.float16, mybir.dt.float8e3

    xt_d = nc.dram_tensor("xt", [128, HALF], f8, kind="ExternalInput")
    w1_d = nc.dram_tensor("w1", [128, D_ENC], f16, kind="ExternalInput")
    w2_d = nc.dram_tensor("w2", [128, C], f16, kind="ExternalInput")
    out_d = nc.dram_tensor("out", [128, SEG_PER_CORE], f32,
                           kind="ExternalOutput")

    with tile.TileContext(nc) as tc:
        with (
            tc.tile_pool(name="consts", bufs=1) as consts,
            tc.tile_pool(name="xin", bufs=3) as xin,
            tc.tile_pool(name="rlp", bufs=3) as rlp,
            tc.tile_pool(name="outp", bufs=2) as outp,
            tc.tile_pool(name="psp", bufs=3, space="PSUM") as psp,
            tc.tile_pool(name="accp", bufs=2, space="PSUM") as accp,
        ):
            w1 = consts.tile([128, D_ENC], f16)
            nc.sync.dma_start(w1[:], w1_d[:])
            w2 = consts.tile([128, C], f16)
            nc.sync.dma_start(w2[:], w2_d[:])

            col0 = 0
            ocol = 0
            pending = None   # mm2 state of previous tile awaiting emission

            class MM2:
                """Deferred classifier-matmul emission for one tile,
                split into sub-groups so emission can interleave with the
                next tile's encoder chunks."""

                def __init__(self, tf, rl, ocol):
                    self.tf, self.rl, self.ocol = tf, rl, ocol
                    gt = tf // J
                    rlh = rl.rearrange("p (h q) -> p h q", h=2)
                    self.acc = accp.tile([128, 512], f32, tag="acc",
                                         name="acc")
                    self.acv = self.acc.rearrange("p (h g) -> p h g", h=2)
                    nfull = tf // CH
                    self.gfull = nfull * (CH // J)
                    self.rem = tf % CH
                    self.rfull = rlh[:, :, 0:nfull * CH].rearrange(
                        "p h (c j g) -> p h c j g", c=nfull, j=J)
                    if self.rem:
                        self.rrem = rlh[:, :, nfull * CH:tf].rearrange(
                            "p h (j g) -> p h j g", j=J)
                    self.gi = 0

                def emit_group(self):
                    if self.gi >= len(K_GROUPS):
                        return
                    k0, k1 = K_GROUPS[self.gi]
                    self.gi += 1
                    for k in range(k0, k1):
                        for s in range(4):
                            if k >= len(J_SETS[s]):
                                continue
                            j = J_SETS[s][k]
                            first = k == 0
                            last = k == len(J_SETS[s]) - 1
                            aout = self.acv[32 * s:32 * s + C, :,
                                            0:self.gfull]
                            nc.tensor.matmul(
                                aout, w2[:], self.rfull[:, :, :, j, :],
                                start=first, stop=(last and self.rem == 0),
                                tile_position=(0, 32 * s))
                            if self.rem:
                                arem = self.acv[32 * s:32 * s + C, :,
                                                self.gfull:
                                                self.gfull + self.rem // J]
                                nc.tensor.matmul(
                                    arem, w2[:], self.rrem[:, :, j, :],
                                    start=False, stop=last,
                                    tile_position=(0, 32 * s))

                def finish(self):
                    while self.gi < len(K_GROUPS):
                        self.emit_group()
                    gt = self.tf // J
                    av = self.acc.rearrange(
                        "p (h g) -> p h g", h=2)[:, :, 0:gt]
                    ob = outp.tile([128, 512], f32, tag="ob")
                    ov = ob[:, 0:2 * gt].rearrange("p (h g) -> p h g", h=2)
                    nc.scalar.copy(ov, av)
                    nc.sync.dma_start(
                        out_d[:, self.ocol:self.ocol + 2 * gt],
                        ob[:, 0:2 * gt])

            for t, tf in enumerate(TFS):
                gt = tf // J
                # ---- load xt tile [128, tf] fp8 (contiguous) ----
                xt = xin.tile([128, 7680], f8, tag="xt")
                nc.sync.dma_start(xt[:, 0:tf], xt_d[:, col0:col0 + tf])

                # relu output, j-major per chunk: rl[p, h*7680 + cb + j*gc + g]
                rl = rlp.tile([128, 2 * 7680], f16, tag="rl")
                rlh = rl.rearrange("p (h q) -> p h q", h=2)

                chunks = [CH] * (tf // CH) + ([tf % CH] if tf % CH else [])
                nch = len(chunks)
                # interleave points: spread pending-tile mm2 groups evenly
                if pending is not None:
                    emit_after = set(
                        round((i + 1) * nch / (len(K_GROUPS) + 1)) - 1
                        for i in range(len(K_GROUPS)))
                else:
                    emit_after = set()
                cb = 0
                for ci, ch in enumerate(chunks):
                    gc = ch // J
                    # PSUM pair tensor: bank0 = A chunk, bank1 = B chunk
                    pp = psp.tile([128, 1024], f32, tag="pp")
                    nc.tensor.matmul(pp[:, 0:ch], w1[0:64, :],
                                     xt[0:64, cb:cb + ch])
                    nc.tensor.matmul(pp[:, 512:512 + ch], w1[64:128, :],
                                     xt[64:128, cb:cb + ch])

                    # relu PSUM -> SBUF fp16; scatter on the PSUM *read* side
                    pin = pp.rearrange("p (h q) -> p h q", h=2)[
                        :, :, 0:ch].rearrange("p h (g j) -> p h j g", j=J)
                    rout = rlh[:, :, cb:cb + ch].rearrange(
                        "p h (j g) -> p h j g", j=J)
                    # VectorE (0.96GHz) carries ~3.4us more pure work than
                    # ScalarE (1.2GHz, but also the drains): shift 3 mid-
                    # kernel chunks to ScalarE, and flip the last tile's
                    # parity so the final relu lands on VectorE while
                    # ScalarE finishes with the drain.
                    use_scalar = ci % 2 == (1 if t == len(TFS) - 1 else 0)
                    if t in (4, 6, 8) and ci == 15:
                        use_scalar = True
                    if use_scalar:
                        nc.scalar.activation(rout, pin,
                                             mybir.ActivationFunctionType.Relu)
                    else:
                        nc.vector.tensor_scalar_max(rout, pin, 0.0)
                    if ci in emit_after and pending is not None:
                        pending.emit_group()
                    cb += ch

                if pending is not None:
                    pending.finish()
                pending = MM2(tf, rl, ocol)
                col0 += tf
                ocol += 2 * gt
            pending.finish()

    nc.compile()
    return nc


def kernel(x: np.ndarray, Wloc: np.ndarray, W: np.ndarray) -> np.ndarray:
    if "nc" not in _CACHE:
        _CACHE["nc"] = _build_kernel()
    nc = _CACHE["nc"]

    x = np.asarray(x, dtype=np.float32)
    # pack per-core transposed fp8 inputs: [8, 128, HALF]
    xp = x.reshape(N_CORES, 2, HALF, D_IN).transpose(0, 1, 3, 2)
    xp = np.ascontiguousarray(xp).astype(ml_dtypes.float8_e3m4)
    xp = xp.reshape(N_CORES, 128, HALF)

    w1 = np.ascontiguousarray(
        np.concatenate([Wloc.T, Wloc.T], axis=0), dtype=np.float16)  # [128,128]
    w2 = np.ascontiguousarray((W / float(J)).T, dtype=np.float16)    # [128,10]

    in_maps = [{"xt": xp[c], "w1": w1, "w2": w2} for c in range(N_CORES)]
    res = run_bass_kernel_spmd(nc, in_maps, core_ids=list(range(N_CORES)))
    _CACHE["exec_time_ns"] = res.exec_time_ns
    _CACHE["trace"] = res.instructions_and_trace

    # host: sum the 4 PE column-group strips, then reorder segments
    out = np.empty((L // J, C), dtype=np.float32)
    for c in range(N_CORES):
        oc = res.results[c]["out"]  # [128, 5000]
        strips = oc[0:10] + oc[32:42] + oc[64:74] + oc[96:106]  # [10, 5000]
        ocol = 0
        gbase = 0
        base = c * SEG_PER_CORE
        for tf in TFS:
            gt = tf // J
            blk = strips[:, ocol:ocol + 2 * gt].reshape(C, 2, gt)
            out[base + gbase:base + gbase + gt] = blk[:, 0].T
            out[base + HALF // J + gbase:base + HALF // J + gbase + gt] = blk[:, 1].T
            ocol += 2 * gt
            gbase += gt
    return out


# revision 26
# speedup vs baseline: 1.0404x; 1.0404x over previous
"""Trainium2 Bass kernel for segment-reduce classifier.

Reference computation:
    local = relu(x @ Wloc.T)            # [L, 128]
    feats = local.reshape(-1, 30, 128).mean(1)   # [L/30, 128]
    out   = feats @ W.T                 # [L/30, 10]

Strategy (8 NeuronCores, data-parallel on rows):
  - Each core gets R = L/8 = 150000 rows, host-transposed, fp8(e3m4)-cast,
    packed as xt [128, 75000]: partitions 0-63 = x_shard[:75000].T ("A"
    half), partitions 64-127 = x_shard[75000:].T ("B" half).  fp8-e3m4
    halves the input DMA (9.6MB/core) vs fp16 and streams through the PE
    at 16-bit speed; 4 mantissa bits keep median rel err ~3.5e-3.
  - matmul1: lhsT = Wloc.T stacked twice [128, 128] fp16 (mixed dtype with
    fp8 rhs); two K=64 matmuls via PE row-groups (tile_position rows 0/64)
    produce localT [128enc, rows] 480-row chunks in PSUM.
  - relu PSUM -> SBUF fp16 with contiguous writes, j-scatter on the PSUM
    read side (PSUM fp32 source is 1 elem/lane/cyc regardless of stride),
    split between ScalarE and VectorE.  This evacuation is the kernel's
    hard floor: 150k lane-elems/core through the only two engines with a
    PSUM read port.
  - mean-pool + classifier fused: 30 accumulating matmuls per tile (one per
    within-segment offset j, rhs j-major) -> pooling is free PSUM
    accumulation. M=10 packed 4x into PE column-groups (tile_position
    (0,32s)); the 4 strips are summed on the host.  mm2 matmuls are
    emitted in sub-groups of ~8 interleaved between mm1 chunks: same-mode
    bursts amortize the PE tiling-mode-switch drain (measured 322ns/mm in
    1-bursts vs 61ns/mm in 8-bursts) while staying short enough not to
    starve the relu pipeline.
  - Tapered tile schedule: small first tiles so the PE starts ~2us after
    launch instead of waiting for a 2MB DMA; small last tile to shrink the
    drain tail.  Output DMA per tile, overlapped with compute.
"""

import numpy as np
import ml_dtypes

import concourse.bacc as bacc
import concourse.bass as bass
import concourse.tile as tile
from concourse import mybir
from concourse.bass_utils import run_bass_kernel_spmd

# Problem constants (hardcoded per harness contract)
L, D_IN, D_ENC, C, J = 1200000, 64, 128, 10, 30
N_CORES = 8
R = L // N_CORES          # rows per core = 150000
HALF = R // 2             # 75000 cols per half-stream
CH = 480                  # chunk rows (16 segments) per matmul
# tapered tile schedule (cols per half); sum = 75000, all %30 == 0
TFS = [960, 1920, 3840] + [7680] * 8 + [4440, 2400]
SEG_PER_CORE = R // J     # 5000
# j-subsets for the 4 PE column-group strips of the classifier matmul
J_SETS = [list(range(0, 8)), list(range(8, 16)),
          list(range(16, 23)), list(range(23, 30))]
# k-step groups for interleaved mm2 emission (pairs of k -> 8 matmuls each)
K_GROUPS = [(0, 2), (2, 4), (4, 6), (6, 8)]

_CACHE = {}


def _build_kernel():
    nc = bacc.Bacc("TRN2", target_bir_lowering=False, debug=False,
                   num_devices=N_CORES)
    f32, f16, f8 = mybir.dt.float32, mybir.dt.float16, mybir.dt.float8e3

    xt_d = nc.dram_tensor("xt", [128, HALF], f8, kind="ExternalInput")
    w1_d = nc.dram_tensor("w1", [128, D_ENC], f16, kind="ExternalInput")
    w2_d = nc.dram_tensor("w2", [128, C], f16, kind="ExternalInput")
    out_d = nc.dram_tensor("out", [128, SEG_PER_CORE], f32,
                           kind="ExternalOutput")

    with tile.TileContext(nc) as tc:
        with (
            tc.tile_pool(name="consts", bufs=1) as consts,
            tc.tile_pool(name="xin", bufs=3) as xin,
            tc.tile_pool(name="rlp", bufs=3) as rlp,
            tc.tile_pool(name="outp", bufs=2) as outp,
            tc.tile_pool(name="psp", bufs=3, space="PSUM") as psp,
            tc.tile_pool(name="accp", bufs=2, space="PSUM") as accp,
        ):
            w1 = consts.tile([128, D_ENC], f16)
            nc.sync.dma_start(w1[:], w1_d[:])
            w2 = consts.tile([128, C], f16)
            nc.sync.dma_start(w2[:], w2_d[:])

            col0 = 0
            ocol = 0
            pending = None   # mm2 state of previous tile awaiting emission

            class MM2:
                """Deferred classifier-matmul emission for one tile,
                split into sub-groups so emission can interleave with the
                next tile's encoder chunks."""

                def __init__(self, tf, rl, ocol):
                    self.tf, self.rl, self.ocol = tf, rl, ocol
                    gt = tf // J
                    rlh = rl.rearrange("p (h q) -> p h q", h=2)
                    self.acc = accp.tile([128, 512], f32, tag="acc",
                                         name="acc")
                    self.acv = self.acc.rearrange("p (h g) -> p h g", h=2)
                    nfull = tf // CH
                    self.gfull = nfull * (CH // J)
                    self.rem = tf % CH
                    self.rfull = rlh[:, :, 0:nfull * CH].rearrange(
                        "p h (c j g) -> p h c j g", c=nfull, j=J)
                    if self.rem:
                        self.rrem = rlh[:, :, nfull * CH:tf].rearrange(
                            "p h (j g) -> p h j g", j=J)
                    self.gi = 0

                def emit_group(self):
                    if self.gi >= len(K_GROUPS):
                        return
                    k0, k1 = K_GROUPS[self.gi]
                    self.gi += 1
                    for k in range(k0, k1):
                        for s in range(4):
                            if k >= len(J_SETS[s]):
                                continue
                            j = J_SETS[s][k]
                            first = k == 0
                            last = k == len(J_SETS[s]) - 1
                            aout = self.acv[32 * s:32 * s + C, :,
                                            0:self.gfull]
                            nc.tensor.matmul(
                                aout, w2[:], self.rfull[:, :, :, j, :],
                                start=first, stop=(last and self.rem == 0),
                                tile_position=(0, 32 * s))
                            if self.rem:
                                arem = self.acv[32 * s:32 * s + C, :,
                                                self.gfull:
                                                self.gfull + self.rem // J]
                                nc.tensor.matmul(
                                    arem, w2[:], self.rrem[:, :, j, :],
                                    start=False, stop=last,
                                    tile_position=(0, 32 * s))

                def finish(self):
                    while self.gi < len(K_GROUPS):
                        self.emit_group()
                    gt = self.tf // J
                    av = self.acc.rearrange(
                        "p (h g) -> p h g", h=2)[:, :, 0:gt]
                    ob = outp.tile([128, 512], f32, tag="ob")
                    ov = ob[:, 0:2 * gt].rearrange("p (h g) -> p h g", h=2)
                    nc.scalar.copy(ov, av)
                    nc.sync.dma_start(
                        out_d[:, self.ocol:self.ocol + 2 * gt],
                        ob[:, 0:2 * gt])

            for t, tf in enumerate(TFS):
                gt = tf // J
                # ---- load xt tile [128, tf] fp8 (contiguous) ----
                xt = xin.tile([128, 7680], f8, tag="xt")
                nc.sync.dma_start(xt[:, 0:tf], xt_d[:, col0:col0 + tf])

                # relu output, j-major per chunk: rl[p, h*7680 + cb + j*gc + g]
                rl = rlp.tile([128, 2 * 7680], f16, tag="rl")
                rlh = rl.rearrange("p (h q) -> p h q", h=2)

                chunks = [CH] * (tf // CH) + ([tf % CH] if tf % CH else [])
                nch = len(chunks)
                # interleave points: spread pending-tile mm2 groups evenly
                if pending is not None:
                    emit_after = set(
                        round((i + 1) * nch / (len(K_GROUPS) + 1)) - 1
                        for i in range(len(K_GROUPS)))
                else:
                    emit_after = set()
                cb = 0
                for ci, ch in enumerate(chunks):
                    gc = ch // J
                    # PSUM pair tensor: bank0 = A chunk, bank1 = B chunk
                    pp = psp.tile([128, 1024], f32, tag="pp")
                    nc.tensor.matmul(pp[:, 0:ch], w1[0:64, :],
                                     xt[0:64, cb:cb + ch])
                    nc.tensor.matmul(pp[:, 512:512 + ch], w1[64:128, :],
                                     xt[64:128, cb:cb + ch])

                    # relu PSUM -> SBUF fp16; scatter on the PSUM *read* side
                    pin = pp.rearrange("p (h q) -> p h q", h=2)[
                        :, :, 0:ch].rearrange("p h (g j) -> p h j g", j=J)
                    rout = rlh[:, :, cb:cb + ch].rearrange(
                        "p h (j g) -> p h j g", j=J)
                    if ci % 2 == 0:
                        nc.scalar.activation(rout, pin,
                                             mybir.ActivationFunctionType.Relu)
                    else:
                        nc.vector.tensor_scalar_max(rout, pin, 0.0)
                    if ci in emit_after and pending is not None:
                        pending.emit_group()
                    cb += ch

                if pending is not None:
                    pending.finish()
                pending = MM2(tf, rl, ocol)
                col0 += tf
                ocol += 2 * gt
            pending.finish()

    nc.compile()
    return nc


def kernel(x: np.ndarray, Wloc: np.ndarray, W: np.ndarray) -> np.ndarray:
    if "nc" not in _CACHE:
        _CACHE["nc"] = _build_kernel()
    nc = _CACHE["nc"]

    x = np.asarray(x, dtype=np.float32)
    # pack per-core transposed fp8 inputs: [8, 128, HALF]
    xp = x.reshape(N_CORES, 2, HALF, D_IN).transpose(0, 1, 3, 2)
    xp = np.ascontiguousarray(xp).astype(ml_dtypes.float8_e3m4)
    xp = xp.reshape(N_CORES, 128, HALF)

    w1 = np.ascontiguousarray(
        np.concatenate([Wloc.T, Wloc.T], axis=0), dtype=np.float16)  # [128,128]
    w2 = np.ascontiguousarray((W / float(J)).T, dtype=np.float16)    # [128,10]

    in_maps = [{"xt": xp[c], "w1": w1, "w2": w2} for c in range(N_CORES)]
    res = run_bass_kernel_spmd(nc, in_maps, core_ids=list(range(N_CORES)))
    _CACHE["exec_time_ns"] = res.exec_time_ns
    _CACHE["trace"] = res.instructions_and_trace

    # host: sum the 4 PE column-group strips, then reorder segments
    out = np.empty((L // J, C), dtype=np.float32)
    for c in range(N_CORES):
        oc = res.results[c]["out"]  # [128, 5000]
        strips = oc[0:10] + oc[32:42] + oc[64:74] + oc[96:106]  # [10, 5000]
        ocol = 0
        gbase = 0
        base = c * SEG_PER_CORE
        for tf in TFS:
            gt = tf // J
            blk = strips[:, ocol:ocol + 2 * gt].reshape(C, 2, gt)
            out[base + gbase:base + gbase + gt] = blk[:, 0].T
            out[base + HALF // J + gbase:base + HALF // J + gbase + gt] = blk[:, 1].T
            ocol += 2 * gt
            gbase += gt
    return out
